# revision 31
# baseline (speedup 1.0000x reference)
"""Bidirectional attention kernel for Trainium2 (8 NeuronCores, data-parallel over batch).

Math per example (B=32, L1=L2=512, D=1024):
    sim = v1 @ v2^T                                  [512, 512]
    attn1 = softmax_j(sim + v2maskbias)              (mask v2 cols)
    attn2 = softmax_i(sim + v1maskbias)              (mask v1 rows)
    out1  = (attn1 @ v2) zeroed at v1-masked rows    [512, 1024]
    out2  = (attn2^T @ v1) zeroed at v2-masked rows  [512, 1024]

Device strategy (4 examples per core), all-16-bit datapath:
  - sim matmul operands in fp16 (host pre-transposed); PSUM fp32. fp16
    logits keep softmax ties stable (bf16 does not: 9e-2 rel err).
  - e1 numerators fp16 (range (0,1]); e2 numerators bf16 (range up to
    e^60 from the global-max-bound trick, needs fp32 exponent range).
  - attend rhs: v2 natural fp16 (out1), v1 natural bf16 (out2). Outputs
    bf16, unpacked/upcast on host.
  - Host packs each per-example tensor into a single [128, x] row-major
    region so every load/store is ONE large DMA (DMA issue instructions
    cost ~650ns each on the issuing engine and serialize).
  - e1 -> e1ji transpose on the PE (fp16 transposes run 1 cyc/row, half
    the fp32 cost); PSUM->SBUF copies balanced across ACT and DVE
    (gpsimd/Pool cannot touch PSUM); store issues on gpsimd.
  - Row softmax stats ride the EXP activation accumulator; 1/sum and
    mask-zeroing fold into PSUM->SBUF output copies (per-partition
    scale) which are spread across ACT/DVE/Pool to keep all three off
    the critical path. Column sums for attn2 via tiny ones-matmuls.
  - 1-example software-pipeline skew: sim+softmax of example e issue
    before the attend matmuls of example e-1 so the PE never waits on
    the softmax stats chain and stays at full p-state.
"""

import numpy as np

B, L, D = 32, 512, 1024
NCORES = 8
EPC = B // NCORES  # examples per core
NB = L // 128      # 128-row blocks per L
ND = D // 128      # 128-row chunks per D (transposed layouts)
NDC = D // 512     # 512-col halves per D

_CACHE = {}
LAST_RESULTS = None


def _build_nc():
    from contextlib import ExitStack
    import concourse.bacc as bacc
    import concourse.tile as tile
    import concourse.mybir as mybir
    import concourse.bass_isa as bass_isa

    f32 = mybir.dt.float32
    f16 = mybir.dt.float16
    bf16 = mybir.dt.bfloat16
    EXP = mybir.ActivationFunctionType.Exp
    COPY = mybir.ActivationFunctionType.Copy
    ADD = mybir.AluOpType.add
    MIN = mybir.AluOpType.min
    MAX = mybir.AluOpType.max
    AXX = mybir.AxisListType.X

    nc = bacc.Bacc("TRN2", target_bir_lowering=False, debug=False, num_devices=NCORES)
    # packed layouts: one [128, x] row-major region per example per tensor
    v1td = nc.dram_tensor("v1t", [EPC * 128, ND * L], f16, kind="ExternalInput")
    v2td = nc.dram_tensor("v2t", [EPC * 128, ND * L], f16, kind="ExternalInput")
    v2nd = nc.dram_tensor("v2n", [EPC * 128, NB * D], f16, kind="ExternalInput")
    v1nd = nc.dram_tensor("v1n", [EPC * 128, NB * D], bf16, kind="ExternalInput")
    b2d = nc.dram_tensor("b2r", [EPC * 128, L], f32, kind="ExternalInput")
    cmd = nc.dram_tensor("cm", [128, 2 * EPC * NB], f32, kind="ExternalInput")
    bcd = nc.dram_tensor("bcol", [128, EPC * NB], f32, kind="ExternalInput")
    idd = nc.dram_tensor("idh", [128, 128], f16, kind="ExternalInput")
    ond = nc.dram_tensor("ones2", [128, 2], bf16, kind="ExternalInput")
    o1d = nc.dram_tensor("o1", [EPC * 128, NB * D], bf16, kind="ExternalOutput")
    o2d = nc.dram_tensor("o2", [EPC * 128, NB * D], bf16, kind="ExternalOutput")
    v1ta, v2ta, v2na, v1na = v1td.ap(), v2td.ap(), v2nd.ap(), v1nd.ap()
    o1a, o2a = o1d.ap(), o2d.ap()

    with ExitStack() as ctx:
        tc = ctx.enter_context(tile.TileContext(nc))
        const = ctx.enter_context(tc.tile_pool(name="const", bufs=1))
        pv = ctx.enter_context(tc.tile_pool(name="pv", bufs=1))
        pvt = ctx.enter_context(tc.tile_pool(name="pvt", bufs=1))
        pe_ = ctx.enter_context(tc.tile_pool(name="pe", bufs=1))
        pst = ctx.enter_context(tc.tile_pool(name="pst", bufs=1))
        pbb = ctx.enter_context(tc.tile_pool(name="pbb", bufs=1))
        pav = ctx.enter_context(tc.tile_pool(name="pav", bufs=1))
        pps = ctx.enter_context(tc.tile_pool(name="pps", bufs=1, space="PSUM"))

        ident = const.tile([128, 128], f16)
        nc.sync.dma_start(out=ident, in_=idd.ap())
        cms = const.tile([128, 2 * EPC * NB], f32)
        nc.sync.dma_start(out=cms, in_=cmd.ap())
        bcs = const.tile([128, EPC * NB], f32)
        nc.sync.dma_start(out=bcs, in_=bcd.ap())
        onesr = const.tile([128, 2], bf16)
        nc.sync.dma_start(out=onesr, in_=ond.ap())

        st = [dict() for _ in range(EPC)]

        def stage_load(e):
            s = st[e]
            # one [128, 8*512] tile per transposed tensor, filled by 4
            # quarter-DMAs (2 chunks each) so the first sim matmuls start
            # ~1.5us after launch instead of waiting for the full 1MB
            # All loads on sync's DMA queue. The first quarter-pair gates the
            # first sim matmul, so it goes first; b2 (needed only when the
            # first sim PSUM block completes) rides after it.
            v1T = pvt.tile([128, ND * L], f16, tag="v1T", bufs=2, name=f"v1T_{e}")
            v2T = pvt.tile([128, ND * L], f16, tag="v2T", bufs=2, name=f"v2T_{e}")
            b2bc0 = pbb.tile([128, L], f32, tag="b2", bufs=2, name=f"b2bc_{e}")
            nc.sync.dma_start(out=b2bc0, in_=b2d.ap()[e * 128 : (e + 1) * 128, :])
            QW = 2 * L  # quarter width: 2 chunks
            for q in range(4):
                nc.sync.dma_start(out=v1T[:, q * QW : (q + 1) * QW],
                                  in_=v1ta[e * 128 : (e + 1) * 128, q * QW : (q + 1) * QW])
                nc.sync.dma_start(out=v2T[:, q * QW : (q + 1) * QW],
                                  in_=v2ta[e * 128 : (e + 1) * 128, q * QW : (q + 1) * QW])
            s["v1T"] = v1T
            s["v2T"] = v2T
            s["b2bc"] = b2bc0
            v2n = pv.tile([128, NB * D], f16, tag="v2n", bufs=2, name=f"v2n_{e}")
            nc.sync.dma_start(out=v2n, in_=v2na[e * 128 : (e + 1) * 128, :])
            v1n = pv.tile([128, NB * D], bf16, tag="v1n", bufs=2, name=f"v1n_{e}")
            nc.sync.dma_start(out=v1n, in_=v1na[e * 128 : (e + 1) * 128, :])
            s["v2n"] = v2n
            s["v1n"] = v1n

        def stage_sim(e):
            s = st[e]
            m1nt = pst.tile([128, NB], f32, tag="m1nt", bufs=2, name=f"m1nt_{e}")
            s1t = pst.tile([128, NB], f32, tag="s1t", bufs=2, name=f"s1t_{e}")
            s["mk"], s["e1"], s["e2"] = [], [], []
            for ib in range(NB):
                ps = pps.tile([128, L], f32, tag="sim", bufs=2)
                for c in range(ND):
                    nc.tensor.matmul(
                        ps,
                        s["v1T"][:, c * L + ib * 128 : c * L + (ib + 1) * 128],
                        s["v2T"][:, c * L : (c + 1) * L],
                        start=(c == 0),
                        stop=(c == ND - 1),
                    )
                mk = pe_.tile([128, L], f32, tag="mk", bufs=2 * NB, name=f"mk_{e}_{ib}")
                nc.vector.tensor_add(mk, ps, s["b2bc"])
                # m1n = -rowmax(mk): the e1 exp bias, negated in one op
                nc.vector.tensor_reduce(m1nt[:, ib : ib + 1], mk, axis=AXX, op=MAX,
                                        negate=True)
                e1 = pe_.tile([128, L], f16, tag="e1", bufs=2 * NB, name=f"e1_{e}_{ib}")
                nc.scalar.activation(out=e1, in_=mk, func=EXP,
                                     bias=m1nt[:, ib : ib + 1], scale=1.0,
                                     accum_out=s1t[:, ib : ib + 1])
                s["mk"].append(mk)
                s["e1"].append(e1)
            # gm = global max = -min(m1n); all-reduce across partitions
            gmx = pst.tile([128, 1], f32, tag="gmx", bufs=2, name=f"gmx_{e}")
            nc.vector.tensor_reduce(gmx, m1nt, axis=AXX, op=MIN, negate=True)
            gmr = pst.tile([128, 1], f32, tag="gmr", bufs=2, name=f"gmr_{e}")
            nc.gpsimd.partition_all_reduce(gmr, gmx, 128, bass_isa.ReduceOp.max)
            # bias = 60 - gm keeps e2 numerators in normal fp32/bf16 range
            gmn = pst.tile([128, 1], f32, tag="gmn", bufs=2, name=f"gmn_{e}")
            nc.vector.tensor_scalar(gmn, gmr, -1.0, 60.0, op0=mybir.AluOpType.mult,
                                    op1=ADD)
            comb2 = pst.tile([128, NB], f32, tag="comb2", bufs=2, name=f"comb2_{e}")
            nc.vector.tensor_scalar_add(comb2, bcs[:, e * NB : e * NB + NB], gmn)
            r1t = pst.tile([128, NB], f32, tag="r1t", bufs=2, name=f"r1t_{e}")
            nc.vector.reciprocal(out=r1t, in_=s1t)
            sc1t = pst.tile([128, NB], f32, tag="sc1t", bufs=2, name=f"sc1t_{e}")
            nc.vector.tensor_mul(sc1t, r1t, cms[:, e * NB : e * NB + NB])
            s["sc1t"] = sc1t
            # e2 = exp(mk + b1col - gm + 60); b2row term cancels per-column
            for ib in range(NB):
                e2 = pe_.tile([128, L], bf16, tag="e2", bufs=2 * NB, name=f"e2_{e}_{ib}")
                nc.scalar.activation(out=e2, in_=s["mk"][ib], func=EXP,
                                     bias=comb2[:, ib : ib + 1], scale=1.0)
                s["e2"].append(e2)

        def stage_finA(e):
            # transpose e1 into [j,i] lhsT layout (fp16 PE transposes).
            # Emitted BEFORE sim(e+1) so the DVE copies don't queue behind
            # the next example's softmax work (e1 deps are long satisfied).
            s = st[e]
            e1ji = pe_.tile([128, NB * L], f16, tag="e1ji", bufs=2, name=f"e1ji_{e}")
            for jb in range(NB):
                ps = pps.tile([128, L], f16, tag="pte", bufs=2, name=f"pt1_{e}_{jb}")
                for ib in range(NB):
                    nc.tensor.transpose(
                        ps[:, ib * 128 : (ib + 1) * 128],
                        s["e1"][ib][:, jb * 128 : (jb + 1) * 128],
                        ident,
                    )
                nc.vector.tensor_copy(e1ji[:, jb * L : (jb + 1) * L], ps)
            s["e1ji"] = e1ji

        def stage_finB(e):
            s = st[e]
            e1ji = s["e1ji"]
            # ---- out1[i,d] = sum_j e1[j,i] v2[j,d] / s1, masked rows zeroed
            for ib in range(NB):
                av = pav.tile([128, D], bf16, tag="av1", bufs=3)
                for dc in range(NDC):
                    ps = pps.tile([128, 512], f32, tag="att", bufs=3)
                    for jb in range(NB):
                        nc.tensor.matmul(
                            ps,
                            e1ji[:, jb * L + ib * 128 : jb * L + (ib + 1) * 128],
                            s["v2n"][:, jb * D + dc * 512 : jb * D + (dc + 1) * 512],
                            start=(jb == 0),
                            stop=(jb == NB - 1),
                        )
                    dst = av[:, dc * 512 : (dc + 1) * 512]
                    if dc == 0:
                        nc.scalar.activation(out=dst, in_=ps, func=COPY,
                                             scale=s["sc1t"][:, ib : ib + 1])
                    else:
                        nc.vector.tensor_scalar_mul(dst, ps, s["sc1t"][:, ib : ib + 1])
                nc.gpsimd.dma_start(
                    out=o1a[e * 128 : (e + 1) * 128, ib * D : (ib + 1) * D], in_=av)
            # ---- s2 column sums via ones-matmuls on e2 tiles
            pss = pps.tile([128, 2 * NB], f32, tag="pss", bufs=1, name=f"pss_{e}")
            for jb in range(NB):
                for ib in range(NB):
                    nc.tensor.matmul(pss[:, 2 * jb : 2 * jb + 2],
                                     s["e2"][ib][:, jb * 128 : (jb + 1) * 128], onesr,
                                     start=(ib == 0), stop=(ib == NB - 1))
            s2t = pst.tile([128, NB], f32, tag="s2t", bufs=2, name=f"s2t_{e}")
            nc.vector.tensor_scalar_add(s2t, pss[:, 0 : 2 * NB : 2], 1.0e-36)
            r2t = pst.tile([128, NB], f32, tag="r2t", bufs=2, name=f"r2t_{e}")
            nc.vector.reciprocal(out=r2t, in_=s2t)
            sc2t = pst.tile([128, NB], f32, tag="sc2t", bufs=2, name=f"sc2t_{e}")
            nc.vector.tensor_mul(sc2t, r2t, cms[:, EPC * NB + e * NB : EPC * NB + e * NB + NB])
            # ---- out2[j,d] = sum_i e2[i,j] v1[i,d] / s2, masked rows zeroed
            for jb in range(NB):
                av = pav.tile([128, D], bf16, tag="av2", bufs=3)
                for dc in range(NDC):
                    ps = pps.tile([128, 512], f32, tag="att", bufs=3)
                    for ib in range(NB):
                        nc.tensor.matmul(
                            ps,
                            s["e2"][ib][:, jb * 128 : (jb + 1) * 128],
                            s["v1n"][:, ib * D + dc * 512 : ib * D + (dc + 1) * 512],
                            start=(ib == 0),
                            stop=(ib == NB - 1),
                        )
                    dst = av[:, dc * 512 : (dc + 1) * 512]
                    if dc == 0:
                        nc.vector.tensor_scalar_mul(dst, ps, sc2t[:, jb : jb + 1])
                    else:
                        nc.scalar.activation(out=dst, in_=ps, func=COPY,
                                             scale=sc2t[:, jb : jb + 1])
                    if e == EPC - 1:
                        # drain the kernel tail: store each half as soon as
                        # its copy lands instead of waiting for the block
                        nc.gpsimd.dma_start(
                            out=o2a[e * 128 : (e + 1) * 128,
                                    jb * D + dc * 512 : jb * D + (dc + 1) * 512],
                            in_=dst)
                if e != EPC - 1:
                    nc.gpsimd.dma_start(
                        out=o2a[e * 128 : (e + 1) * 128, jb * D : (jb + 1) * D], in_=av)
            st[e] = {}

        stage_load(0)
        stage_sim(0)
        for e in range(1, EPC):
            stage_load(e)
            stage_finA(e - 1)
            stage_sim(e)
            stage_finB(e - 1)
        stage_finA(EPC - 1)
        stage_finB(EPC - 1)

    nc.compile()
    return nc


def get_nc():
    if "nc" not in _CACHE:
        _CACHE["nc"] = _build_nc()
    return _CACHE["nc"]


def _host_prep(v1, v2, v1_mask, v2_mask):
    """Build per-core input maps (packed per-example layouts) from full inputs."""
    import ml_dtypes

    bf16 = ml_dtypes.bfloat16
    v1 = np.asarray(v1, dtype=np.float32)
    v2 = np.asarray(v2, dtype=np.float32)
    v1_mask = np.asarray(v1_mask).astype(bool)
    v2_mask = np.asarray(v2_mask).astype(bool)

    def pack_t(x):  # [EPC, L, D] -> [EPC*128, ND*L]; row p = concat chunk rows
        return np.ascontiguousarray(
            x.transpose(0, 2, 1).reshape(EPC, ND, 128, L).transpose(0, 2, 1, 3)
            .reshape(EPC * 128, ND * L))

    def pack_n(x):  # [EPC, L, D] -> [EPC*128, NB*D]; row p = concat block rows
        return np.ascontiguousarray(
            x.reshape(EPC, NB, 128, D).transpose(0, 2, 1, 3).reshape(EPC * 128, NB * D))

    in_maps = []
    for k in range(NCORES):
        sl = slice(EPC * k, EPC * (k + 1))
        v1c, v2c = v1[sl], v2[sl]
        m1 = v1_mask[sl]
        m2 = v2_mask[sl]
        b1 = np.where(m1, np.float32(-1e30), np.float32(0.0)).astype(np.float32)
        b2 = np.where(m2, np.float32(-1e30), np.float32(0.0)).astype(np.float32)
        bcol = np.ascontiguousarray(b1.reshape(EPC, NB, 128).transpose(2, 0, 1).reshape(128, EPC * NB))
        b2rep = np.repeat(b2[:, None, :], 128, axis=1).reshape(EPC * 128, L)
        k1 = (~m1).astype(np.float32).reshape(EPC, NB, 128).transpose(2, 0, 1).reshape(128, EPC * NB)
        k2 = (~m2).astype(np.float32).reshape(EPC, NB, 128).transpose(2, 0, 1).reshape(128, EPC * NB)
        in_maps.append(
            {
                "v1t": pack_t(v1c).astype(np.float16),
                "v2t": pack_t(v2c).astype(np.float16),
                "v2n": pack_n(v2c).astype(np.float16),
                "v1n": pack_n(v1c).astype(bf16),
                "b2r": np.ascontiguousarray(b2rep),
                "bcol": bcol,
                "ones2": np.ones((128, 2), bf16),
                "cm": np.ascontiguousarray(np.concatenate([k1, k2], axis=1)),
                "idh": np.eye(128, dtype=np.float16),
            }
        )
    return in_maps


def kernel(v1, v2, v1_mask, v2_mask):
    global LAST_RESULTS
    from concourse.bass_utils import run_bass_kernel_spmd

    nc = get_nc()
    in_maps = _host_prep(v1, v2, v1_mask, v2_mask)
    res = run_bass_kernel_spmd(nc, in_maps, list(range(NCORES)))
    LAST_RESULTS = res

    def unpack(name):
        parts = []
        for k in range(NCORES):
            arr = res.results[k][name].astype(np.float32)
            parts.append(arr.reshape(EPC, 128, NB, D).transpose(0, 2, 1, 3).reshape(EPC, L, D))
        return np.concatenate(parts, axis=0)

    return unpack("o1"), unpack("o2")


# revision 33
# speedup vs baseline: 1.0772x; 1.0772x over previous
"""Bidirectional attention kernel for Trainium2 (8 NeuronCores, data-parallel over batch).

Math per example (B=32, L1=L2=512, D=1024):
    sim = v1 @ v2^T                                  [512, 512]
    attn1 = softmax_j(sim + v2maskbias)              (mask v2 cols)
    attn2 = softmax_i(sim + v1maskbias)              (mask v1 rows)
    out1  = (attn1 @ v2) zeroed at v1-masked rows    [512, 1024]
    out2  = (attn2^T @ v1) zeroed at v2-masked rows  [512, 1024]

Device strategy (4 examples per core), all-16-bit datapath:
  - sim matmul operands in fp16 (host pre-transposed); PSUM fp32. fp16
    logits keep softmax ties stable (bf16 does not: 9e-2 rel err).
  - e1 numerators fp16 (range (0,1]); e2 numerators bf16 (range up to
    e^60 from the global-max-bound trick, needs fp32 exponent range).
  - attend rhs: v2 natural fp16 (out1), v1 natural bf16 (out2). Outputs
    bf16, unpacked/upcast on host.
  - Host packs each per-example tensor into a single [128, x] row-major
    region so every load/store is ONE large DMA (DMA issue instructions
    cost ~650ns each on the issuing engine and serialize).
  - e1 -> e1ji transpose on the PE (fp16 transposes run 1 cyc/row, half
    the fp32 cost); PSUM->SBUF copies balanced across ACT and DVE
    (gpsimd/Pool cannot touch PSUM); store issues on gpsimd.
  - Row softmax stats ride the EXP activation accumulator; 1/sum and
    mask-zeroing fold into PSUM->SBUF output copies (per-partition
    scale) which are spread across ACT/DVE/Pool to keep all three off
    the critical path. Column sums for attn2 via tiny ones-matmuls.
  - 1-example software-pipeline skew: sim+softmax of example e issue
    before the attend matmuls of example e-1 so the PE never waits on
    the softmax stats chain and stays at full p-state.
"""

import numpy as np

B, L, D = 32, 512, 1024
NCORES = 8
EPC = B // NCORES  # examples per core
NB = L // 128      # 128-row blocks per L
ND = D // 128      # 128-row chunks per D (transposed layouts)
NDC = D // 512     # 512-col halves per D

_CACHE = {}
LAST_RESULTS = None


def _build_nc():
    from contextlib import ExitStack
    import concourse.bacc as bacc
    import concourse.tile as tile
    import concourse.mybir as mybir
    import concourse.bass_isa as bass_isa

    f32 = mybir.dt.float32
    f16 = mybir.dt.float16
    bf16 = mybir.dt.bfloat16
    EXP = mybir.ActivationFunctionType.Exp
    COPY = mybir.ActivationFunctionType.Copy
    ADD = mybir.AluOpType.add
    MIN = mybir.AluOpType.min
    MAX = mybir.AluOpType.max
    AXX = mybir.AxisListType.X

    nc = bacc.Bacc("TRN2", target_bir_lowering=False, debug=False, num_devices=NCORES)
    # packed layouts: one [128, x] row-major region per example per tensor
    v1td = nc.dram_tensor("v1t", [EPC * 128, ND * L], f16, kind="ExternalInput")
    v2td = nc.dram_tensor("v2t", [EPC * 128, ND * L], f16, kind="ExternalInput")
    v2nd = nc.dram_tensor("v2n", [EPC * 128, NB * D], f16, kind="ExternalInput")
    v1nd = nc.dram_tensor("v1n", [EPC * 128, NB * D], bf16, kind="ExternalInput")
    b2d = nc.dram_tensor("b2r", [EPC * 128, L], f32, kind="ExternalInput")
    cmd = nc.dram_tensor("cm", [128, 2 * EPC * NB], f32, kind="ExternalInput")
    bcd = nc.dram_tensor("bcol", [128, EPC * NB], f32, kind="ExternalInput")
    idd = nc.dram_tensor("idh", [128, 128], f16, kind="ExternalInput")
    ond = nc.dram_tensor("ones2", [128, 2], bf16, kind="ExternalInput")
    o1d = nc.dram_tensor("o1", [EPC * 128, NB * D], bf16, kind="ExternalOutput")
    o2d = nc.dram_tensor("o2", [EPC * 128, NB * D], bf16, kind="ExternalOutput")
    v1ta, v2ta, v2na, v1na = v1td.ap(), v2td.ap(), v2nd.ap(), v1nd.ap()
    o1a, o2a = o1d.ap(), o2d.ap()

    with ExitStack() as ctx:
        tc = ctx.enter_context(tile.TileContext(nc))
        const = ctx.enter_context(tc.tile_pool(name="const", bufs=1))
        pv = ctx.enter_context(tc.tile_pool(name="pv", bufs=1))
        pvt = ctx.enter_context(tc.tile_pool(name="pvt", bufs=1))
        pe_ = ctx.enter_context(tc.tile_pool(name="pe", bufs=1))
        pst = ctx.enter_context(tc.tile_pool(name="pst", bufs=1))
        pbb = ctx.enter_context(tc.tile_pool(name="pbb", bufs=1))
        pav = ctx.enter_context(tc.tile_pool(name="pav", bufs=1))
        pps = ctx.enter_context(tc.tile_pool(name="pps", bufs=1, space="PSUM"))

        ident = const.tile([128, 128], f16)
        nc.sync.dma_start(out=ident, in_=idd.ap())
        cms = const.tile([128, 2 * EPC * NB], f32)
        nc.sync.dma_start(out=cms, in_=cmd.ap())
        bcs = const.tile([128, EPC * NB], f32)
        nc.sync.dma_start(out=bcs, in_=bcd.ap())
        onesr = const.tile([128, 2], bf16)
        nc.sync.dma_start(out=onesr, in_=ond.ap())

        st = [dict() for _ in range(EPC)]

        def stage_load(e):
            s = st[e]
            # one [128, 8*512] tile per transposed tensor, filled by 4
            # quarter-DMAs (2 chunks each) so the first sim matmuls start
            # ~1.5us after launch instead of waiting for the full 1MB
            # All loads on sync's DMA queue. The first quarter-pair gates the
            # first sim matmul, so it goes first; b2 (needed only when the
            # first sim PSUM block completes) rides after it.
            v1T = pvt.tile([128, ND * L], f16, tag="v1T", bufs=2, name=f"v1T_{e}")
            v2T = pvt.tile([128, ND * L], f16, tag="v2T", bufs=2, name=f"v2T_{e}")
            b2bc0 = pbb.tile([128, L], f32, tag="b2", bufs=2, name=f"b2bc_{e}")
            # first quarter as two single-chunk pieces: the very first sim
            # matmul only needs chunk 0 of each operand, and the DMA engine
            # ramps slowly at kernel start
            for p in range(2):
                nc.sync.dma_start(out=v1T[:, p * L : (p + 1) * L],
                                  in_=v1ta[e * 128 : (e + 1) * 128, p * L : (p + 1) * L])
                nc.sync.dma_start(out=v2T[:, p * L : (p + 1) * L],
                                  in_=v2ta[e * 128 : (e + 1) * 128, p * L : (p + 1) * L])
            QW = 2 * L  # quarter width: 2 chunks
            for q in range(1, 4):
                nc.sync.dma_start(out=v1T[:, q * QW : (q + 1) * QW],
                                  in_=v1ta[e * 128 : (e + 1) * 128, q * QW : (q + 1) * QW])
                nc.sync.dma_start(out=v2T[:, q * QW : (q + 1) * QW],
                                  in_=v2ta[e * 128 : (e + 1) * 128, q * QW : (q + 1) * QW])
                if q == 1:
                    nc.sync.dma_start(out=b2bc0, in_=b2d.ap()[e * 128 : (e + 1) * 128, :])
            s["v1T"] = v1T
            s["v2T"] = v2T
            s["b2bc"] = b2bc0
            v2n = pv.tile([128, NB * D], f16, tag="v2n", bufs=2, name=f"v2n_{e}")
            nc.sync.dma_start(out=v2n, in_=v2na[e * 128 : (e + 1) * 128, :])
            v1n = pv.tile([128, NB * D], bf16, tag="v1n", bufs=2, name=f"v1n_{e}")
            nc.sync.dma_start(out=v1n, in_=v1na[e * 128 : (e + 1) * 128, :])
            s["v2n"] = v2n
            s["v1n"] = v1n

        def stage_sim(e):
            s = st[e]
            m1nt = pst.tile([128, NB], f32, tag="m1nt", bufs=2, name=f"m1nt_{e}")
            s1t = pst.tile([128, NB], f32, tag="s1t", bufs=2, name=f"s1t_{e}")
            s["mk"], s["e1"], s["e2"] = [], [], []
            for ib in range(NB):
                ps = pps.tile([128, L], f32, tag="sim", bufs=2)
                for c in range(ND):
                    nc.tensor.matmul(
                        ps,
                        s["v1T"][:, c * L + ib * 128 : c * L + (ib + 1) * 128],
                        s["v2T"][:, c * L : (c + 1) * L],
                        start=(c == 0),
                        stop=(c == ND - 1),
                    )
                mk = pe_.tile([128, L], f32, tag="mk", bufs=2 * NB, name=f"mk_{e}_{ib}")
                nc.vector.tensor_add(mk, ps, s["b2bc"])
                # m1n = -rowmax(mk): the e1 exp bias, negated in one op
                nc.vector.tensor_reduce(m1nt[:, ib : ib + 1], mk, axis=AXX, op=MAX,
                                        negate=True)
                e1 = pe_.tile([128, L], f16, tag="e1", bufs=2 * NB, name=f"e1_{e}_{ib}")
                nc.scalar.activation(out=e1, in_=mk, func=EXP,
                                     bias=m1nt[:, ib : ib + 1], scale=1.0,
                                     accum_out=s1t[:, ib : ib + 1])
                s["mk"].append(mk)
                s["e1"].append(e1)
            # gm = global max = -min(m1n); all-reduce across partitions
            gmx = pst.tile([128, 1], f32, tag="gmx", bufs=2, name=f"gmx_{e}")
            nc.vector.tensor_reduce(gmx, m1nt, axis=AXX, op=MIN, negate=True)
            gmr = pst.tile([128, 1], f32, tag="gmr", bufs=2, name=f"gmr_{e}")
            nc.gpsimd.partition_all_reduce(gmr, gmx, 128, bass_isa.ReduceOp.max)
            # bias = 60 - gm keeps e2 numerators in normal fp32/bf16 range
            gmn = pst.tile([128, 1], f32, tag="gmn", bufs=2, name=f"gmn_{e}")
            nc.vector.tensor_scalar(gmn, gmr, -1.0, 60.0, op0=mybir.AluOpType.mult,
                                    op1=ADD)
            comb2 = pst.tile([128, NB], f32, tag="comb2", bufs=2, name=f"comb2_{e}")
            nc.vector.tensor_scalar_add(comb2, bcs[:, e * NB : e * NB + NB], gmn)
            r1t = pst.tile([128, NB], f32, tag="r1t", bufs=2, name=f"r1t_{e}")
            nc.vector.reciprocal(out=r1t, in_=s1t)
            sc1t = pst.tile([128, NB], f32, tag="sc1t", bufs=2, name=f"sc1t_{e}")
            nc.vector.tensor_mul(sc1t, r1t, cms[:, e * NB : e * NB + NB])
            s["sc1t"] = sc1t
            # e2 = exp(mk + b1col - gm + 60); b2row term cancels per-column
            for ib in range(NB):
                e2 = pe_.tile([128, L], bf16, tag="e2", bufs=2 * NB, name=f"e2_{e}_{ib}")
                nc.scalar.activation(out=e2, in_=s["mk"][ib], func=EXP,
                                     bias=comb2[:, ib : ib + 1], scale=1.0)
                s["e2"].append(e2)

        def stage_finA(e):
            # transpose e1 into [j,i] lhsT layout (fp16 PE transposes).
            # Emitted BEFORE sim(e+1) so the DVE copies don't queue behind
            # the next example's softmax work (e1 deps are long satisfied).
            s = st[e]
            e1ji = pe_.tile([128, NB * L], f16, tag="e1ji", bufs=2, name=f"e1ji_{e}")
            for jb in range(NB):
                ps = pps.tile([128, L], f16, tag="pte", bufs=2, name=f"pt1_{e}_{jb}")
                for ib in range(NB):
                    nc.tensor.transpose(
                        ps[:, ib * 128 : (ib + 1) * 128],
                        s["e1"][ib][:, jb * 128 : (jb + 1) * 128],
                        ident,
                    )
                if jb % 2 == 0:
                    nc.vector.tensor_copy(e1ji[:, jb * L : (jb + 1) * L], ps)
                else:
                    nc.scalar.copy(e1ji[:, jb * L : (jb + 1) * L], ps)
            s["e1ji"] = e1ji

        def stage_finB(e):
            s = st[e]
            e1ji = s["e1ji"]
            # ---- out1[i,d] = sum_j e1[j,i] v2[j,d] / s1, masked rows zeroed
            for ib in range(NB):
                av = pav.tile([128, D], bf16, tag="av1", bufs=3)
                for dc in range(NDC):
                    ps = pps.tile([128, 512], f32, tag="att", bufs=3)
                    for jb in range(NB):
                        nc.tensor.matmul(
                            ps,
                            e1ji[:, jb * L + ib * 128 : jb * L + (ib + 1) * 128],
                            s["v2n"][:, jb * D + dc * 512 : jb * D + (dc + 1) * 512],
                            start=(jb == 0),
                            stop=(jb == NB - 1),
                        )
                    dst = av[:, dc * 512 : (dc + 1) * 512]
                    if dc == 0:
                        nc.scalar.activation(out=dst, in_=ps, func=COPY,
                                             scale=s["sc1t"][:, ib : ib + 1])
                    else:
                        nc.vector.tensor_scalar_mul(dst, ps, s["sc1t"][:, ib : ib + 1])
                nc.gpsimd.dma_start(
                    out=o1a[e * 128 : (e + 1) * 128, ib * D : (ib + 1) * D], in_=av)
            # ---- s2 column sums via ones-matmuls on e2 tiles
            pss = pps.tile([128, 2 * NB], f32, tag="pss", bufs=1, name=f"pss_{e}")
            for jb in range(NB):
                for ib in range(NB):
                    nc.tensor.matmul(pss[:, 2 * jb : 2 * jb + 2],
                                     s["e2"][ib][:, jb * 128 : (jb + 1) * 128], onesr,
                                     start=(ib == 0), stop=(ib == NB - 1))
            s2t = pst.tile([128, NB], f32, tag="s2t", bufs=2, name=f"s2t_{e}")
            nc.vector.tensor_scalar_add(s2t, pss[:, 0 : 2 * NB : 2], 1.0e-36)
            r2t = pst.tile([128, NB], f32, tag="r2t", bufs=2, name=f"r2t_{e}")
            nc.vector.reciprocal(out=r2t, in_=s2t)
            sc2t = pst.tile([128, NB], f32, tag="sc2t", bufs=2, name=f"sc2t_{e}")
            nc.vector.tensor_mul(sc2t, r2t, cms[:, EPC * NB + e * NB : EPC * NB + e * NB + NB])
            # ---- out2[j,d] = sum_i e2[i,j] v1[i,d] / s2, masked rows zeroed
            for jb in range(NB):
                av = pav.tile([128, D], bf16, tag="av2", bufs=3)
                for dc in range(NDC):
                    ps = pps.tile([128, 512], f32, tag="att", bufs=3)
                    for ib in range(NB):
                        nc.tensor.matmul(
                            ps,
                            s["e2"][ib][:, jb * 128 : (jb + 1) * 128],
                            s["v1n"][:, ib * D + dc * 512 : ib * D + (dc + 1) * 512],
                            start=(ib == 0),
                            stop=(ib == NB - 1),
                        )
                    dst = av[:, dc * 512 : (dc + 1) * 512]
                    if dc == 0:
                        nc.vector.tensor_scalar_mul(dst, ps, sc2t[:, jb : jb + 1])
                    else:
                        nc.scalar.activation(out=dst, in_=ps, func=COPY,
                                             scale=sc2t[:, jb : jb + 1])
                    if e == EPC - 1:
                        # drain the kernel tail: store each half as soon as
                        # its copy lands instead of waiting for the block
                        nc.gpsimd.dma_start(
                            out=o2a[e * 128 : (e + 1) * 128,
                                    jb * D + dc * 512 : jb * D + (dc + 1) * 512],
                            in_=dst)
                if e != EPC - 1:
                    nc.gpsimd.dma_start(
                        out=o2a[e * 128 : (e + 1) * 128, jb * D : (jb + 1) * D], in_=av)
            st[e] = {}

        stage_load(0)
        stage_sim(0)
        for e in range(1, EPC):
            stage_load(e)
            stage_finA(e - 1)
            stage_sim(e)
            stage_finB(e - 1)
        stage_finA(EPC - 1)
        stage_finB(EPC - 1)

    nc.compile()
    return nc


def get_nc():
    if "nc" not in _CACHE:
        _CACHE["nc"] = _build_nc()
    return _CACHE["nc"]


def _host_prep(v1, v2, v1_mask, v2_mask):
    """Build per-core input maps (packed per-example layouts) from full inputs."""
    import ml_dtypes

    bf16 = ml_dtypes.bfloat16
    v1 = np.asarray(v1, dtype=np.float32)
    v2 = np.asarray(v2, dtype=np.float32)
    v1_mask = np.asarray(v1_mask).astype(bool)
    v2_mask = np.asarray(v2_mask).astype(bool)

    def pack_t(x):  # [EPC, L, D] -> [EPC*128, ND*L]; row p = concat chunk rows
        return np.ascontiguousarray(
            x.transpose(0, 2, 1).reshape(EPC, ND, 128, L).transpose(0, 2, 1, 3)
            .reshape(EPC * 128, ND * L))

    def pack_n(x):  # [EPC, L, D] -> [EPC*128, NB*D]; row p = concat block rows
        return np.ascontiguousarray(
            x.reshape(EPC, NB, 128, D).transpose(0, 2, 1, 3).reshape(EPC * 128, NB * D))

    in_maps = []
    for k in range(NCORES):
        sl = slice(EPC * k, EPC * (k + 1))
        v1c, v2c = v1[sl], v2[sl]
        m1 = v1_mask[sl]
        m2 = v2_mask[sl]
        b1 = np.where(m1, np.float32(-1e30), np.float32(0.0)).astype(np.float32)
        b2 = np.where(m2, np.float32(-1e30), np.float32(0.0)).astype(np.float32)
        bcol = np.ascontiguousarray(b1.reshape(EPC, NB, 128).transpose(2, 0, 1).reshape(128, EPC * NB))
        b2rep = np.repeat(b2[:, None, :], 128, axis=1).reshape(EPC * 128, L)
        k1 = (~m1).astype(np.float32).reshape(EPC, NB, 128).transpose(2, 0, 1).reshape(128, EPC * NB)
        k2 = (~m2).astype(np.float32).reshape(EPC, NB, 128).transpose(2, 0, 1).reshape(128, EPC * NB)
        in_maps.append(
            {
                "v1t": pack_t(v1c).astype(np.float16),
                "v2t": pack_t(v2c).astype(np.float16),
                "v2n": pack_n(v2c).astype(np.float16),
                "v1n": pack_n(v1c).astype(bf16),
                "b2r": np.ascontiguousarray(b2rep),
                "bcol": bcol,
                "ones2": np.ones((128, 2), bf16),
                "cm": np.ascontiguousarray(np.concatenate([k1, k2], axis=1)),
                "idh": np.eye(128, dtype=np.float16),
            }
        )
    return in_maps


def kernel(v1, v2, v1_mask, v2_mask):
    global LAST_RESULTS
    from concourse.bass_utils import run_bass_kernel_spmd

    nc = get_nc()
    in_maps = _host_prep(v1, v2, v1_mask, v2_mask)
    res = run_bass_kernel_spmd(nc, in_maps, list(range(NCORES)))
    LAST_RESULTS = res

    def unpack(name):
        parts = []
        for k in range(NCORES):
            arr = res.results[k][name].astype(np.float32)
            parts.append(arr.reshape(EPC, 128, NB, D).transpose(0, 2, 1, 3).reshape(EPC, L, D))
        return np.concatenate(parts, axis=0)

    return unpack("o1"), unpack("o2")
